# revision 24
# baseline (speedup 1.0000x reference)
# GAT layer kernel for Trainium2 (Bass/Tile), 8 NeuronCores data-parallel.
#
# Problem: B=16, S=64 -> 1024 independent 256-node graphs, F=O=64, H=1.
#   h = x @ W; a_s = h@att_src; a_d = h@att_dst
#   e[i,j] = leaky_relu(a_d[i] + a_s[j], 0.2) masked to (adj[j,i]!=0 | i==j)
#   alpha = softmax_j(e); out = alpha @ h + bias
#
# V2 pipeline ("exp-max + fp16 bit-trick fifth root"):
#   exp(leaky(zeta)) = max(exp(zeta), exp(0.2 zeta)).
#   * PE writes z = a_d_bcast + BIG*(adj|I) - BIG into PSUM (as V1).
#   * ACT does ONE pass: t1 = Exp(z + a_s) PSUM->SBUF fp16 (per-256 block,
#     bias = per-partition a_s). Masked entries underflow to exactly +0.
#   * DVE computes the second branch WITHOUT an exp: for positive fp16,
#     bits(x^0.2) ~= 0.2*bits(x) + 0.8*15360 + delta  (Schraudolph).
#     t1 = exp(zeta + a_s) so t1^0.2 = exp(0.2 zeta)*exp(0.2 a_s) -- the
#     a_s factor comes out right automatically. One 4x-mode tensor_scalar
#     on int16 views + one 2x-mode tensor_tensor max.  delta = -60 tuned
#     numerically (rel err 9e-3 vs 2e-2 budget).
#   * Masked entries: t1 = +0 -> bits 0 -> t2 = CVAL (constant 0.1213).
#     Cancelled EXACTLY by 2 extra matmuls chained into each aggregation
#     PSUM group: lhsT = adj mask fp8 {-1,0} (already in SBUF), rhs =
#     CVAL*[h|a_s|1]. Sign works out since mask rows are -1.
#   * Aggregation gains a ones column -> denominator for free (col 65).
#   * adj and out are host-laid-out contiguous per quad so every DMA moves
#     2KB-contiguous per-partition lines (no sub-512B descriptor penalty).

import os
import numpy as np

B, S, N, F, O = 16, 64, 256, 64, 64
G = B * S                  # 1024 graphs
NCORES = 8
GPC = G // NCORES          # 128 graphs per core
BIG = 16384.0
NEG_SLOPE = 0.2

# fp16 bit-trick constants: bits(t2) = (bits(t1) + S1) * 0.2
BIT_DELTA = -60.0
BIT_S1 = (12288.0 + BIT_DELTA) / 0.2   # 61140
CVAL = float(np.frombuffer(np.uint16(int(12288 + BIT_DELTA)).tobytes(),
                           dtype=np.float16)[0])   # fp16 value of masked t2

# h|a_s copy engine per pair: "act" or "vec" (engine balance knob)
HS_COPY = ("act", "vec")
# SBUF tile-pool depths (pipeline depth across quads)
BUFS_DEEP = 4
BUFS_MID = 4

_CACHE = {}


def _build(with_bias, reps=1):
    import concourse.bass as bass
    import concourse.tile as tile
    import concourse.bacc as bacc
    import concourse.mybir as mybir

    dt = mybir.dt
    f32, f16, i16 = dt.float32, dt.float16, dt.int16
    f8 = dt.float8e5
    AF = mybir.ActivationFunctionType
    ALU = mybir.AluOpType

    nc = bacc.Bacc("TRN2", debug=False)

    # xt is host-preshuffled to the exact per-quad SBUF image [128, 512]
    xT_d = nc.dram_tensor("xt", [GPC // 4, 128, 512], f16,
                          kind="ExternalInput").ap()
    adj_d = nc.dram_tensor("adjm", [GPC // 4, 128, 2048], f8,
                           kind="ExternalInput").ap()
    wvs_d = nc.dram_tensor("wvs", [128, 66], f16, kind="ExternalInput").ap()
    vdb_d = nc.dram_tensor("vdb", [128, 128], f16, kind="ExternalInput").ap()
    idn_d = nc.dram_tensor("idn", [128, 128], f8, kind="ExternalInput").ap()
    if with_bias:
        bias_d = nc.dram_tensor("biasv", [O], f32, kind="ExternalInput").ap()
    out_d = nc.dram_tensor("out", [GPC // 4, 128, 512], f32,
                           kind="ExternalOutput").ap()

    with tile.TileContext(nc) as tc:
        from contextlib import ExitStack
        ctx = ExitStack()
        with ctx:
            consts = ctx.enter_context(tc.tile_pool(name="consts", bufs=1))
            xt_pool = ctx.enter_context(tc.tile_pool(name="xt", bufs=BUFS_DEEP))
            adj_pool = ctx.enter_context(tc.tile_pool(name="adj", bufs=BUFS_DEEP))
            h_pool = ctx.enter_context(tc.tile_pool(name="h", bufs=BUFS_DEEP))
            e_pool = ctx.enter_context(tc.tile_pool(name="e", bufs=BUFS_MID))
            p_pool = ctx.enter_context(tc.tile_pool(name="p", bufs=BUFS_MID))
            o_pool = ctx.enter_context(tc.tile_pool(name="o", bufs=BUFS_DEEP))
            ps_eb = ctx.enter_context(tc.tile_pool(name="ps_eb", bufs=2,
                                                   space="PSUM"))
            ps_h = ctx.enter_context(tc.tile_pool(name="ps_h", bufs=2, space="PSUM"))
            ps_ag = ctx.enter_context(tc.tile_pool(name="ps_ag", bufs=2, space="PSUM"))

            wvs = consts.tile([128, 66], f16)
            nc.sync.dma_start(out=wvs, in_=wvs_d)
            vdb = consts.tile([128, 128], f16)
            nc.sync.dma_start(out=vdb, in_=vdb_d)
            idn = consts.tile([128, 128], f8)
            nc.sync.dma_start(out=idn, in_=idn_d)
            if with_bias:
                bias_sb = consts.tile([128, O], f32)
                bias_b = bass.AP(
                    tensor=bias_d.tensor, offset=bias_d.offset,
                    ap=[[0, 128]] + list(bias_d.ap),
                )
                nc.gpsimd.dma_start(out=bias_sb, in_=bias_b)

            def emit_quad(q):
                # ---- load 4 graphs' xT: parts 0:64 = g0,g1; 64:128 = g2,g3
                xt = xt_pool.tile([128, 512], f16)
                nc.sync.dma_start(out=xt, in_=xT_d[q])
                # ---- adjacency mask, host-prelaid contiguous [128, 2048]
                adjq = adj_pool.tile([128, 2048], f8)
                nc.sync.dma_start(out=adjq, in_=adj_d[q])
                outq = o_pool.tile([128, 512], f32, tag="out")

                for pr in range(2):
                    lo = 64 * pr       # partition base of this pair in xt

                    # ---- h | a_s : one matmul per (graph, node-chunk)
                    psh = ps_h.tile([128, 264], f32)
                    for b in range(4):
                        gl, c = b // 2, b % 2
                        nc.tensor.matmul(
                            out=psh[:, 66 * b: 66 * b + 66],
                            lhsT=xt[lo: lo + 64,
                                    256 * gl + 128 * c: 256 * gl + 128 * c + 128],
                            rhs=wvs[lo: lo + 64],
                            start=True, stop=True,
                        )

                    # hs = [h(64) | a_s | ones] per block, fp16
                    hs = h_pool.tile([128, 264], f16)
                    psh_r = psh.rearrange("p (b c) -> p b c", b=4)
                    hs_r = hs.rearrange("p (b c) -> p b c", b=4)
                    if HS_COPY[pr] == "act":
                        nc.scalar.copy(hs_r[:, :, 0:65], psh_r[:, :, 0:65])
                    else:
                        nc.vector.tensor_copy(hs_r[:, :, 0:65], psh_r[:, :, 0:65])
                    nc.vector.memset(hs_r[:, :, 65:66], 1.0)
                    # corr rhs: CVAL * [h | a_s | 1]  (4x-mode tensor_scalar)
                    h8 = p_pool.tile([128, 264], f16, tag="h8")
                    nc.vector.tensor_scalar(
                        out=h8, in0=hs, scalar1=CVAL, scalar2=None,
                        op0=ALU.mult,
                    )

                    # ---- scores: eb[j, (gl,cj,i)] = a_d[i] + a_s[j]
                    #      + BIG*(adj|I) - BIG   (a_s folded in via PE outer)
                    eb = ps_eb.tile([128, 1024], f32, name="eb")
                    for gl in range(2):
                        xs = xt[lo: lo + 64, 256 * gl: 256 * gl + 256]
                        xs2 = bass.AP(
                            tensor=xs.tensor, offset=xs.offset,
                            ap=[xs.ap[0], [0, 2]] + list(xs.ap[1:]),
                        )
                        nc.tensor.matmul(
                            out=eb[:, 512 * gl: 512 * gl + 512],
                            lhsT=vdb[lo: lo + 64],
                            rhs=xs2,
                            start=True, stop=False,
                        )
                        nc.tensor.matmul(
                            out=eb[:, 512 * gl: 512 * gl + 512],
                            lhsT=idn,
                            rhs=adjq[:, 1024 * pr + 512 * gl:
                                     1024 * pr + 512 * gl + 512],
                            start=False, stop=False,
                        )
                        # a_s[j] broadcast: lhsT = xT slice (f x j), rhs = vs
                        # column stride-0 repeated 256 wide
                        for cj in range(2):
                            vsc = wvs[lo: lo + 64, 64:65]
                            vsb = bass.AP(
                                tensor=vsc.tensor, offset=vsc.offset,
                                ap=[vsc.ap[0], [0, 256]],
                            )
                            nc.tensor.matmul(
                                out=eb[:, 512 * gl + 256 * cj:
                                       512 * gl + 256 * cj + 256],
                                lhsT=xt[lo: lo + 64,
                                        256 * gl + 128 * cj:
                                        256 * gl + 128 * cj + 128],
                                rhs=vsb,
                                start=False, stop=(cj == 1),
                                skip_group_check=True,
                            )

                    # ---- t1 = exp(z): one wide ACT op, PSUM -> SBUF fp16
                    t1 = e_pool.tile([128, 1024], f16, tag="t1")
                    nc.scalar.activation(
                        out=t1, in_=eb, func=AF.Exp, scale=1.0,
                    )

                    # ---- t2 = t1^0.2 via fp16 bit trick (DVE 4x int16)
                    t2 = p_pool.tile([128, 1024], f16, tag="t2")
                    nc.vector.tensor_scalar(
                        out=t2.bitcast(i16), in0=t1.bitcast(i16),
                        scalar1=BIT_S1, scalar2=0.2,
                        op0=ALU.add, op1=ALU.mult,
                    )
                    # ---- p = max(t1, t2) (DVE 2x fp16)
                    p_sb = e_pool.tile([128, 1024], f16, tag="p")
                    nc.vector.tensor_tensor(out=p_sb, in0=t1, in1=t2, op=ALU.max)

                    # ---- aggregation + mask correction + denominator
                    agg = ps_ag.tile([128, 264], f32)
                    for a in range(4):
                        gl, ci = a // 2, a % 2
                        ao = slice(66 * a, 66 * a + 66)
                        for cj in range(2):
                            nc.tensor.matmul(
                                out=agg[:, ao],
                                lhsT=p_sb[:, 512 * gl + 256 * cj + 128 * ci:
                                          512 * gl + 256 * cj + 128 * ci + 128],
                                rhs=hs[:, 66 * (2 * gl + cj):
                                       66 * (2 * gl + cj) + 66],
                                start=(cj == 0), stop=False,
                            )
                        for cj in range(2):
                            nc.tensor.matmul(
                                out=agg[:, ao],
                                lhsT=adjq[:, 1024 * pr + 512 * gl + 256 * cj
                                          + 128 * ci:
                                          1024 * pr + 512 * gl + 256 * cj
                                          + 128 * ci + 128],
                                rhs=h8[:, 66 * (2 * gl + cj):
                                       66 * (2 * gl + cj) + 66],
                                start=False, stop=(cj == 1),
                            )

                    # ---- normalize (and bias)
                    agg_r = agg.rearrange("p (a c) -> p a c", a=4)
                    rs = o_pool.tile([128, 4], f32, tag="rs")
                    nc.vector.reciprocal(
                        out=rs.rearrange("p (a c) -> p a c", a=4),
                        in_=agg_r[:, :, 65:66],
                    )
                    rs_b = bass.AP(
                        tensor=rs.tensor, offset=rs.offset,
                        ap=[rs.ap[0], [1, 4], [0, 64]],
                    )
                    out_r = outq[:, 256 * pr: 256 * pr + 256].rearrange(
                        "p (a c) -> p a c", a=4)
                    nc.vector.tensor_tensor(
                        out=out_r, in0=agg_r[:, :, 0:64], in1=rs_b,
                        op=ALU.mult,
                    )
                    if with_bias:
                        bias_b4 = bass.AP(
                            tensor=bias_sb.tensor, offset=bias_sb.offset,
                            ap=[bias_sb.ap[0], [0, 4], [1, 64]],
                        )
                        nc.vector.tensor_tensor(
                            out=out_r, in0=out_r, in1=bias_b4, op=ALU.add,
                        )

                nc.gpsimd.dma_start(out=out_d[q], in_=outq)

            def body(_iv=None):
                for q in range(GPC // 4):
                    emit_quad(q)

            if reps == 1:
                body()
            else:
                with tc.For_i(0, reps, 1) as _i:
                    body()
    nc.compile()
    return nc


def kernel(x, adj, W, att_src, att_dst, bias):
    from concourse.bass_utils import run_bass_kernel_spmd

    x = np.asarray(x, dtype=np.float32)
    adj = np.asarray(adj)
    W = np.asarray(W, dtype=np.float32)
    att_src = np.asarray(att_src, dtype=np.float32)
    att_dst = np.asarray(att_dst, dtype=np.float32)
    bias = np.asarray(bias, dtype=np.float32)

    # ---- host-side marshalling (layout prep only)
    # per-quad SBUF image: [q, part=(gp, f), free=(gl, i)]
    xg = np.ascontiguousarray(
        x.reshape(G // 4, 2, 2, N, F)                    # [q, gp, gl, n, f]
        .transpose(0, 1, 4, 2, 3)                        # [q, gp, f, gl, n]
        .reshape(G // 4, 128, 512)).astype(np.float16)
    ar = np.arange(N)
    import ml_dtypes
    adjm = (adj.reshape(G, N, N) == 0).astype(np.int8)
    np.negative(adjm, out=adjm)                          # {-1, 0}
    adjm[:, ar, ar] = 0                                  # self loops always kept
    # contiguous per-quad image [q, p, (pr, gl, cj, i)]
    adjm4 = np.ascontiguousarray(
        adjm.reshape(G // 4, 2, 2, 2, 128, N)            # [q, pr, gl, cj, p, i]
        .transpose(0, 4, 1, 2, 3, 5)                     # [q, p, pr, gl, cj, i]
        .reshape(G // 4, 128, 2048)).astype(ml_dtypes.float8_e5m2)

    vs = W @ att_src.reshape(-1)                         # [F]
    vd = W @ att_dst.reshape(-1)                         # [F]
    wvs = np.zeros((128, 66), np.float16)
    wvs[0:64, 0:64] = W
    wvs[64:128, 0:64] = W
    wvs[0:64, 64] = vs
    wvs[64:128, 64] = vs
    vdb = np.zeros((128, 128), np.float16)
    vdb[0:64] = np.repeat(vd[:, None], 128, axis=1)
    vdb[64:128] = vdb[0:64]
    idn = (np.eye(128, dtype=np.float32) * BIG).astype(ml_dtypes.float8_e5m2)

    with_bias = bool(np.any(bias))
    key = ("gat2", with_bias)
    if key not in _CACHE:
        _CACHE[key] = _build(with_bias)
    nc = _CACHE[key]

    qpc = GPC // 4
    in_maps = []
    for c in range(NCORES):
        m = {
            "xt": np.ascontiguousarray(xg[c * qpc:(c + 1) * qpc]),
            "adjm": np.ascontiguousarray(adjm4[c * qpc:(c + 1) * qpc]),
            "wvs": wvs,
            "vdb": vdb,
            "idn": idn,
        }
        if with_bias:
            m["biasv"] = bias
        in_maps.append(m)

    trace = os.environ.get("GAT_TRACE", "0") == "1"
    res = run_bass_kernel_spmd(
        nc, in_maps, core_ids=list(range(NCORES)), trace=trace,
    )
    global LAST_EXEC_NS, _LAST_NC, _LAST_IN_MAPS
    LAST_EXEC_NS = res.exec_time_ns
    _LAST_NC = nc
    _LAST_IN_MAPS = in_maps

    # out image [q, p, (pr, gl, ci, o)] -> [B, S, N, O]
    out4 = np.concatenate([r["out"] for r in res.results], axis=0)
    out = (out4.reshape(G // 4, 128, 2, 2, 2, 64)
           .transpose(0, 2, 3, 4, 1, 5)                  # [q, pr, gl, ci, p, o]
           .reshape(B, S, N, O))
    return np.ascontiguousarray(out)


LAST_EXEC_NS = None
